# revision 14
# baseline (speedup 1.0000x reference)
"""Trainium2 Bass kernel for the sparse-conv network (nn_ExampleNet).

Parity-packed (space-to-depth) formulation: activations live in SBUF as
[128 partitions = 64ch x column-parity], so every 3x3 conv is 6 matmuls of
[K=128, M=128] per 4 output rows (75% PE util) using VALID-style quadrant
weights with alternating block alignment between layers.  All activations/
weights bf16 (fp32 PSUM accumulation), masks bf16.  8-way SPMD: 4 batches x
2 row-halves; 8 chunks of 16 h3-rows per core.  convT emits h4 directly in
parity layout (3 matmuls / 2 rows); conv5 consumes it with delta in {0,1}.
The single h4 column v=512 is dropped on device (psum width) and final
output column 510 is recomputed on the host.
"""
from contextlib import ExitStack

import numpy as np
import ml_dtypes

import concourse.bacc as bacc
import concourse.mybir as mybir
import concourse.tile as tile
from concourse.bass_utils import run_bass_kernel_spmd

F32 = mybir.dt.float32
BF16 = mybir.dt.bfloat16
RELU = mybir.ActivationFunctionType.Relu
ADD = mybir.AluOpType.add
MAX = mybir.AluOpType.max

B, H, W = 4, 256, 256
NCH = 8            # chunks per core
R3 = 16            # h3 rows produced per chunk
NBF = ml_dtypes.bfloat16

_CACHE = {}


def _np_conv3(xp, w):
    Bc, Hc, Wc, Ci = xp.shape
    xpad = np.zeros((Bc, Hc + 2, Wc + 2, Ci), np.float32)
    xpad[:, 1:-1, 1:-1] = xp
    out = np.zeros((Bc, Hc, Wc, w.shape[3]), np.float32)
    for dy in range(3):
        for dx in range(3):
            out += xpad[:, dy:dy + Hc, dx:dx + Wc] @ w[dy, dx]
    return out


def _np_convT(xp, w):
    Bc, Hc, Wc, Ci = xp.shape
    out = np.zeros((Bc, 2 * Hc + 1, 2 * Wc + 1, w.shape[3]), np.float32)
    for dy in range(3):
        for dx in range(3):
            out[:, dy:dy + 2 * Hc:2, dx:dx + 2 * Wc:2] += xp @ w[dy, dx]
    return out


def _quad_weights(w):
    """conv with SAME/VALID quadrant packing: mats[(dy, delta)] [128,128]:
    [cin+64p, cout+64q] = w[dy, 2*delta+p-q] when 0<=dx<=2."""
    cin, cout = w.shape[2], w.shape[3]
    out = np.zeros((128, 6, 128), np.float32)
    for dy in range(3):
        for delta in range(2):
            for p in range(2):
                for q in range(2):
                    dx = 2 * delta + p - q
                    if 0 <= dx <= 2:
                        out[64 * p:64 * p + cin, dy * 2 + delta,
                            64 * q:64 * q + cout] = w[dy, dx]
    return out


def _conv1_weights(w1):
    out = np.zeros((128, 8, 128), np.float32)
    for op in range(2):
        for pd in range(2):
            for delta in range(2):
                idx = op * 4 + pd * 2 + delta
                for s in range(2):
                    dy = 2 * pd + s - op
                    if not (0 <= dy <= 2):
                        continue
                    for p in range(2):
                        for q in range(2):
                            dx = 2 * delta + p - q
                            if 0 <= dx <= 2:
                                out[64 * s + 32 * p:64 * s + 32 * p + 32, idx,
                                    64 * q:64 * q + 64] = w1[dy, dx]
    return out


def _convT_weights(wt_eff):
    out = np.zeros((128, 3, 128), np.float32)
    for dy in range(3):
        out[64:128, dy, 0:64] = wt_eff[dy, 0]
        out[64:128, dy, 64:128] = wt_eff[dy, 1]
        out[0:64, dy, 0:64] = wt_eff[dy, 2]
    return out


def _host_prep(features, coors, w1, b1, w2, b2, w3, b3, wt, bt, w5, b5):
    f32 = np.float32
    bi, yi, xi = coors[:, 0], coors[:, 1], coors[:, 2]
    flat = (bi.astype(np.int64) * H + yi) * W + xi
    dense = np.zeros((B * H * W, 32), f32)
    for c in range(32):
        dense[:, c] = np.bincount(flat, weights=features[:, c],
                                  minlength=B * H * W)
    dense = dense.reshape(B, H, W, 32)
    occ = np.bincount(flat, minlength=B * H * W).reshape(B, H, W) > 0
    m0p = np.zeros((B, H + 2, W + 2), bool)
    m0p[:, 1:-1, 1:-1] = occ
    m1 = np.zeros((B, H, W), bool)
    for dy in range(3):
        for dx in range(3):
            m1 |= m0p[:, dy:dy + H, dx:dx + W]
    m4 = np.zeros((B, 2 * H + 1, 2 * W + 1), bool)
    for dy in range(3):
        for dx in range(3):
            m4[:, dy:dy + 2 * H - 1:2, dx:dx + 2 * W - 1:2] |= m1

    wt_eff = wt[::-1, ::-1]
    wc1 = _conv1_weights(w1).astype(NBF)
    wc2 = _quad_weights(w2).astype(NBF)
    wc3 = _quad_weights(w3).astype(NBF)
    wcT = _convT_weights(wt_eff).astype(NBF)
    wc5 = _quad_weights(w5).astype(NBF)
    biases = np.stack([np.concatenate([b, b]) for b in
                       (b1, b2, b3, bt, b5)], 1).astype(f32)

    in_maps = []
    for core in range(8):
        bb, half = core // 2, core % 2
        A0 = 128 * half

        # xs: rows A0-4 .. A0+131 (136 = 68 pairs), partition (s, p, cin)
        xp = np.zeros((136, 262, 32), f32)
        lo, hi = max(0, A0 - 4), min(H, A0 + 132)
        xp[lo - (A0 - 4):hi - (A0 - 4), 1:W + 1] = dense[bb, lo:hi]
        xs = xp[:, 0:260].reshape(68, 2, 130, 2, 32)
        xs = xs.transpose(1, 3, 4, 0, 2).reshape(128, 68, 130)

        # m1P: rows A0-3 .. A0+130 (134), partition (p, ch)
        mp_ = np.zeros((134, 262), f32)
        lo, hi = max(0, A0 - 3), min(H, A0 + 131)
        mp_[lo - (A0 - 3):hi - (A0 - 3), 1:W + 1] = m1[bb, lo:hi]
        m1P = mp_[:, 0:260].reshape(134, 130, 2).transpose(2, 0, 1)
        m1P = np.broadcast_to(m1P[:, None], (2, 64, 134, 130)).reshape(
            128, 134, 130)

        # m1 plain: rows A0-1 .. A0+128 (130), idx j = col j-1
        mq = np.zeros((130, 258), f32)
        lo, hi = max(0, A0 - 1), min(H, A0 + 129)
        mq[lo - (A0 - 1):hi - (A0 - 1), 1:W + 1] = m1[bb, lo:hi]
        m1pl = np.broadcast_to(mq[None], (64, 130, 258))

        # m4P: h4 rows U0 .. U0+257, partition (c, ch), block x: v=2x+c
        U0 = 2 * A0
        mr = np.zeros((258, 512), f32)
        lo, hi = U0, min(513, U0 + 258)
        mr[0:hi - lo] = m4[bb, lo:hi, 0:512]
        m4P = mr.reshape(258, 256, 2).transpose(2, 0, 1)
        m4P = np.broadcast_to(m4P[:, None], (2, 64, 258, 256)).reshape(
            128, 258, 256)

        in_maps.append(dict(
            xs=np.ascontiguousarray(xs.astype(NBF)),
            m1P=np.ascontiguousarray(m1P.astype(NBF)),
            m1pl=np.ascontiguousarray(m1pl.astype(NBF)),
            m4P=np.ascontiguousarray(m4P.astype(NBF)),
            wc1=wc1, wc2=wc2, wc3=wc3, wcT=wcT, wc5=wc5, biases=biases,
        ))

    # ---- host column-510 patch data: strip conv for final col 510
    relu = lambda a: np.maximum(a, 0)
    xstrip = dense[:, :, 242:256]                      # cols 242..255
    m1s = m1[:, :, 242:256].astype(f32)[..., None]
    h = relu(_np_conv3(xstrip, w1) + b1) * m1s
    h = relu(_np_conv3(h, w2) + b2) * m1s
    h = relu(_np_conv3(h, w3) + b3) * m1s
    h4s = relu(_np_convT(h, wt_eff) + bt)              # [B, 513, 29, 64]
    m4s = m4[:, :, 484:513].astype(f32)[..., None]
    h4s = h4s * m4s
    col510 = np.zeros((B, 511, 64), f32)
    for dy in range(3):
        for dx in range(3):
            col510 += h4s[:, dy:dy + 511, 26 + dx] @ w5[dy, dx]
    col510 = relu(col510 + b5)
    _CACHE["col510"] = col510
    return in_maps


def _build_program():
    nc = bacc.Bacc("TRN2", target_bir_lowering=False, debug=False,
                   enable_asserts=True, num_devices=8)

    xs_d = nc.dram_tensor("xs", [128, 68, 130], BF16, kind="ExternalInput").ap()
    m1P_d = nc.dram_tensor("m1P", [128, 134, 130], BF16,
                           kind="ExternalInput").ap()
    m1pl_d = nc.dram_tensor("m1pl", [64, 130, 258], BF16,
                            kind="ExternalInput").ap()
    m4P_d = nc.dram_tensor("m4P", [128, 258, 256], BF16,
                           kind="ExternalInput").ap()
    wc1_d = nc.dram_tensor("wc1", [128, 8, 128], BF16, kind="ExternalInput").ap()
    wc2_d = nc.dram_tensor("wc2", [128, 6, 128], BF16, kind="ExternalInput").ap()
    wc3_d = nc.dram_tensor("wc3", [128, 6, 128], BF16, kind="ExternalInput").ap()
    wcT_d = nc.dram_tensor("wcT", [128, 3, 128], BF16, kind="ExternalInput").ap()
    wc5_d = nc.dram_tensor("wc5", [128, 6, 128], BF16, kind="ExternalInput").ap()
    bias_d = nc.dram_tensor("biases", [128, 5], F32, kind="ExternalInput").ap()
    out_d = nc.dram_tensor("out", [128, 256, 256], BF16,
                           kind="ExternalOutput").ap()

    with tile.TileContext(nc) as tc, ExitStack() as ctx:
        wp = ctx.enter_context(tc.tile_pool(name="wp", bufs=1))
        xp = ctx.enter_context(tc.tile_pool(name="xp", bufs=2))
        mp = ctx.enter_context(tc.tile_pool(name="mp", bufs=2))
        hp = ctx.enter_context(tc.tile_pool(name="hp", bufs=1))
        pp = ctx.enter_context(tc.tile_pool(name="pp", bufs=2, space="PSUM"))
        op = ctx.enter_context(tc.tile_pool(name="op", bufs=4))

        w1t = wp.tile([128, 8, 128], BF16, name="w1t")
        w2t = wp.tile([128, 6, 128], BF16, name="w2t")
        w3t = wp.tile([128, 6, 128], BF16, name="w3t")
        wTt = wp.tile([128, 3, 128], BF16, name="wTt")
        w5t = wp.tile([128, 6, 128], BF16, name="w5t")
        bt = wp.tile([128, 5], F32, name="bt")
        nc.sync.dma_start(w1t[:], wc1_d[:])
        nc.sync.dma_start(bt[:], bias_d[:])
        nc.scalar.dma_start(w2t[:], wc2_d[:])
        nc.scalar.dma_start(w3t[:], wc3_d[:])

        def load_chunk(c):
            x_ch = xp.tile([128, 12, 130], BF16, name="x_ch", tag="x")
            nc.sync.dma_start(x_ch[:], xs_d[:, 8 * c:8 * c + 12, :])
            m1P_ch = mp.tile([128, 22, 130], BF16, name="m1P_ch", tag="m1P")
            nc.sync.dma_start(m1P_ch[:], m1P_d[:, 16 * c:16 * c + 22, :])
            m1pl_ch = mp.tile([64, 18, 258], BF16, name="m1pl_ch", tag="m1pl")
            m4_ch = mp.tile([128, 34, 256], BF16, name="m4_ch", tag="m4")
            return x_ch, m1P_ch, m1pl_ch, m4_ch

        def conv1(x_ch, m1P_ch, h1):
            nc.gpsimd.memset(h1[0:64, :, 0:1], 0)
            nc.gpsimd.memset(h1[64:128, :, 128:130], 0)
            for j0 in range(0, 22, 4):
                g = min(4, 22 - j0)
                pc = pp.tile([128, 4, 128], F32, name="p1", tag="p23")
                for r in range(g):
                    o = j0 + r
                    k, opar = o // 2, o % 2
                    i = 0
                    for pd in range(2):
                        for delta in range(2):
                            nc.tensor.matmul(
                                pc[:, r:r + 1, :],
                                w1t[:, opar * 4 + pd * 2 + delta, :],
                                x_ch[:, k + pd, delta:delta + 128],
                                start=(i == 0), stop=(i == 3))
                            i += 1
                nc.scalar.activation(h1[64:128, j0:j0 + g, 0:128],
                                     pc[0:64, 0:g, :], RELU, bias=bt[0:64, 0:1])
                nc.scalar.activation(h1[0:64, j0:j0 + g, 1:129],
                                     pc[64:128, 0:g, :], RELU,
                                     bias=bt[64:128, 0:1])
                nc.vector.tensor_mul(h1[:, j0:j0 + g, :], h1[:, j0:j0 + g, :],
                                     m1P_ch[:, j0:j0 + g, :])

        def conv_mid(inp, wt_, m1P_ch, nrows, bias_ap, moff, h_out):
            nc.gpsimd.memset(h_out[0:64, :, 0:1], 0)
            nc.gpsimd.memset(h_out[64:128, :, 128:130], 0)
            for j0 in range(0, nrows, 4):
                g = min(4, nrows - j0)
                pc = pp.tile([128, 4, 128], F32, name="p23", tag="p23")
                i = 0
                for dy in range(3):
                    for delta in range(2):
                        nc.tensor.matmul(
                            pc[:, 0:g, :], wt_[:, dy * 2 + delta, :],
                            inp[:, j0 + dy:j0 + dy + g, delta:delta + 128],
                            start=(i == 0), stop=(i == 5))
                        i += 1
                nc.scalar.activation(h_out[64:128, j0:j0 + g, 0:128],
                                     pc[0:64, 0:g, :], RELU, bias=bias_ap[0:64])
                nc.scalar.activation(h_out[0:64, j0:j0 + g, 1:129],
                                     pc[64:128, 0:g, :], RELU,
                                     bias=bias_ap[64:128])
                nc.vector.tensor_mul(
                    h_out[:, j0:j0 + g, :], h_out[:, j0:j0 + g, :],
                    m1P_ch[:, j0 + moff:j0 + moff + g, :])

        def conv3_plain(h2, m1pl_ch, h3):
            nc.gpsimd.memset(h3[0:64, :, 0:1], 0)
            for j0 in range(0, 18, 4):
                g = min(4, 18 - j0)
                pc = pp.tile([128, 4, 128], F32, name="p23", tag="p23")
                i = 0
                for dy in range(3):
                    for delta in range(2):
                        nc.tensor.matmul(
                            pc[:, 0:g, :], w3t[:, dy * 2 + delta, :],
                            h2[:, j0 + dy:j0 + dy + g, delta:delta + 128],
                            start=(i == 0), stop=(i == 5))
                        i += 1
                nc.scalar.activation(h3[0:64, j0:j0 + g, 1:257:2],
                                     pc[0:64, 0:g, :], RELU, bias=bt[0:64, 2:3])
                nc.scalar.activation(h3[0:64, j0:j0 + g, 2:258:2],
                                     pc[64:128, 0:g, :], RELU,
                                     bias=bt[64:128, 2:3])
                nc.vector.tensor_mul(h3[0:64, j0:j0 + g, 1:257],
                                     h3[0:64, j0:j0 + g, 1:257],
                                     m1pl_ch[:, j0:j0 + g, 1:257])
                nc.vector.tensor_copy(h3[64:128, j0:j0 + g, 0:256],
                                      h3[0:64, j0:j0 + g, 1:257])

        def convT_part(h3, h4, m4_ch, u0s, last=False):
            for u0 in u0s:
                yl = u0 // 2 + 1
                pc = pp.tile([128, 2, 256], F32, name="pT", tag="pT", bufs=3)
                nc.tensor.matmul(pc[:, 0:1, :], wTt[:, 0, :],
                                 h3[:, yl, 0:256], start=True, stop=False)
                nc.tensor.matmul(pc[:, 0:1, :], wTt[:, 2, :],
                                 h3[:, yl - 1, 0:256], start=False, stop=True)
                nc.tensor.matmul(pc[:, 1:2, :], wTt[:, 1, :],
                                 h3[:, yl, 0:256], start=True, stop=True)
                if last:
                    nc.scalar.activation(h4[:, u0:u0 + 2, 0:256], pc[:],
                                         RELU, bias=bt[:, 3:4])
                else:
                    nc.vector.tensor_scalar(h4[:, u0:u0 + 2, 0:256], pc[:],
                                            bt[:, 3:4], 0.0, ADD, MAX)
                nc.vector.tensor_mul(h4[:, u0:u0 + 2, 0:256],
                                     h4[:, u0:u0 + 2, 0:256],
                                     m4_ch[:, u0:u0 + 2, :])

        def conv5_part(h4, c, jo0s):
            for jo0 in jo0s:
                pc = pp.tile([128, 2, 256], F32, name="p5", tag="p5", bufs=3)
                i = 0
                for dy in range(3):
                    for delta in range(2):
                        nc.tensor.matmul(
                            pc[:], w5t[:, dy * 2 + delta, :],
                            h4[:, jo0 + dy:jo0 + dy + 2, delta:delta + 256],
                            start=(i == 0), stop=(i == 5))
                        i += 1
                out_sb = op.tile([128, 2, 256], BF16, name="out_sb", tag="o")
                nc.scalar.activation(out_sb[:], pc[:], RELU, bias=bt[:, 4:5])
                nc.gpsimd.dma_start(out_d[:, 32 * c + jo0:32 * c + jo0 + 2, :],
                                    out_sb[:])

        prev = None
        for it in range(NCH):
            x_ch, m1P_ch, m1pl_ch, m4_ch = load_chunk(it)
            h1 = hp.tile([128, 22, 130], BF16, name="h1", tag="h1")
            h2 = hp.tile([128, 20, 130], BF16, name="h2", tag="h2")
            h3 = hp.tile([128, 18, 258], BF16, name="h3", tag="h3", bufs=2)
            h4 = hp.tile([128, 34, 258], BF16, name="h4", tag="h4", bufs=2)
            nc.gpsimd.memset(h4[:, :, 256:258], 0)
            conv1(x_ch, m1P_ch, h1)
            if it == 0:
                nc.gpsimd.dma_start(wTt[:], wcT_d[:])
                nc.gpsimd.dma_start(w5t[:], wc5_d[:])
            nc.gpsimd.dma_start(m4_ch[:], m4P_d[:, 32 * it:32 * it + 34, :])
            nc.scalar.dma_start(m1pl_ch[:],
                                m1pl_d[:, 16 * it:16 * it + 18, :])
            if prev is not None:
                h4p, cp = prev
                conv5_part(h4p, cp, range(0, 16, 2))
            conv_mid(h1, w2t, m1P_ch, 20, bt[:, 1:2], 1, h2)
            if prev is not None:
                conv5_part(h4p, cp, range(16, 32, 2))
            conv3_plain(h2, m1pl_ch, h3)
            convT_part(h3, h4, m4_ch, range(0, 34, 2))
            prev = (h4, it)
        h4p, cp = prev
        conv5_part(h4p, cp, range(0, 32, 2))

    nc.compile()
    return nc


def kernel(**inputs):
    features = np.asarray(inputs["features"], np.float32)
    coors = np.asarray(inputs["coors"], np.int32)
    args = [np.asarray(inputs[k], np.float32) for k in
            ("w1", "b1", "w2", "b2", "w3", "b3", "wt", "bt", "w5", "b5")]
    in_maps = _host_prep(features, coors, *args)
    if "nc" not in _CACHE:
        _CACHE["nc"] = _build_program()
    res = run_bass_kernel_spmd(_CACHE["nc"], in_maps,
                               core_ids=list(range(8)), trace=False)
    full = np.zeros((B, 511, 511, 64), np.float32)
    for core in range(8):
        o = np.asarray(res.results[core]["out"], dtype=np.float32)
        o = o.reshape(2, 64, 256, 256)            # [q, ch, row, blk]
        rows = np.zeros((256, 512, 64), np.float32)
        rows[:, 0::2] = o[0].transpose(1, 2, 0)
        rows[:, 1::2] = o[1].transpose(1, 2, 0)
        bb, half = core // 2, core % 2
        if half == 0:
            full[bb, 0:256] = rows[:, 0:511]
        else:
            full[bb, 256:511] = rows[0:255, 0:511]
    full[:, :, 510, :] = _CACHE["col510"]
    return full


# revision 15
# speedup vs baseline: 1.0221x; 1.0221x over previous
"""Trainium2 Bass kernel for the sparse-conv network (nn_ExampleNet).

Parity-packed (space-to-depth) formulation: activations live in SBUF as
[128 partitions = 64ch x column-parity], so every 3x3 conv is 6 matmuls of
[K=128, M=128] per 4 output rows (75% PE util) using VALID-style quadrant
weights with alternating block alignment between layers.  All activations/
weights bf16 (fp32 PSUM accumulation), masks bf16.  8-way SPMD: 4 batches x
2 row-halves; 8 chunks of 16 h3-rows per core.  convT emits h4 directly in
parity layout (3 matmuls / 2 rows); conv5 consumes it with delta in {0,1}.
The single h4 column v=512 is dropped on device (psum width) and final
output column 510 is recomputed on the host.
"""
from contextlib import ExitStack

import numpy as np
import ml_dtypes

import concourse.bacc as bacc
import concourse.mybir as mybir
import concourse.tile as tile
from concourse.bass_utils import run_bass_kernel_spmd

F32 = mybir.dt.float32
BF16 = mybir.dt.bfloat16
RELU = mybir.ActivationFunctionType.Relu
ADD = mybir.AluOpType.add
MAX = mybir.AluOpType.max

B, H, W = 4, 256, 256
NCH = 8            # chunks per core
R3 = 16            # h3 rows produced per chunk
NBF = ml_dtypes.bfloat16

_CACHE = {}


def _np_conv3(xp, w):
    Bc, Hc, Wc, Ci = xp.shape
    xpad = np.zeros((Bc, Hc + 2, Wc + 2, Ci), np.float32)
    xpad[:, 1:-1, 1:-1] = xp
    out = np.zeros((Bc, Hc, Wc, w.shape[3]), np.float32)
    for dy in range(3):
        for dx in range(3):
            out += xpad[:, dy:dy + Hc, dx:dx + Wc] @ w[dy, dx]
    return out


def _np_convT(xp, w):
    Bc, Hc, Wc, Ci = xp.shape
    out = np.zeros((Bc, 2 * Hc + 1, 2 * Wc + 1, w.shape[3]), np.float32)
    for dy in range(3):
        for dx in range(3):
            out[:, dy:dy + 2 * Hc:2, dx:dx + 2 * Wc:2] += xp @ w[dy, dx]
    return out


def _quad_weights(w):
    """conv with SAME/VALID quadrant packing: mats[(dy, delta)] [128,128]:
    [cin+64p, cout+64q] = w[dy, 2*delta+p-q] when 0<=dx<=2."""
    cin, cout = w.shape[2], w.shape[3]
    out = np.zeros((128, 6, 128), np.float32)
    for dy in range(3):
        for delta in range(2):
            for p in range(2):
                for q in range(2):
                    dx = 2 * delta + p - q
                    if 0 <= dx <= 2:
                        out[64 * p:64 * p + cin, dy * 2 + delta,
                            64 * q:64 * q + cout] = w[dy, dx]
    return out


def _conv1_weights(w1):
    out = np.zeros((128, 8, 128), np.float32)
    for op in range(2):
        for pd in range(2):
            for delta in range(2):
                idx = op * 4 + pd * 2 + delta
                for s in range(2):
                    dy = 2 * pd + s - op
                    if not (0 <= dy <= 2):
                        continue
                    for p in range(2):
                        for q in range(2):
                            dx = 2 * delta + p - q
                            if 0 <= dx <= 2:
                                out[64 * s + 32 * p:64 * s + 32 * p + 32, idx,
                                    64 * q:64 * q + 64] = w1[dy, dx]
    return out


def _convT_weights(wt_eff):
    out = np.zeros((128, 3, 128), np.float32)
    for dy in range(3):
        out[64:128, dy, 0:64] = wt_eff[dy, 0]
        out[64:128, dy, 64:128] = wt_eff[dy, 1]
        out[0:64, dy, 0:64] = wt_eff[dy, 2]
    return out


def _host_prep(features, coors, w1, b1, w2, b2, w3, b3, wt, bt, w5, b5):
    f32 = np.float32
    bi, yi, xi = coors[:, 0], coors[:, 1], coors[:, 2]
    flat = (bi.astype(np.int64) * H + yi) * W + xi
    dense = np.zeros((B * H * W, 32), f32)
    for c in range(32):
        dense[:, c] = np.bincount(flat, weights=features[:, c],
                                  minlength=B * H * W)
    dense = dense.reshape(B, H, W, 32)
    occ = np.bincount(flat, minlength=B * H * W).reshape(B, H, W) > 0
    m0p = np.zeros((B, H + 2, W + 2), bool)
    m0p[:, 1:-1, 1:-1] = occ
    m1 = np.zeros((B, H, W), bool)
    for dy in range(3):
        for dx in range(3):
            m1 |= m0p[:, dy:dy + H, dx:dx + W]
    m4 = np.zeros((B, 2 * H + 1, 2 * W + 1), bool)
    for dy in range(3):
        for dx in range(3):
            m4[:, dy:dy + 2 * H - 1:2, dx:dx + 2 * W - 1:2] |= m1

    wt_eff = wt[::-1, ::-1]
    wc1 = _conv1_weights(w1).astype(NBF)
    wc2 = _quad_weights(w2).astype(NBF)
    wc3 = _quad_weights(w3).astype(NBF)
    wcT = _convT_weights(wt_eff).astype(NBF)
    wc5 = _quad_weights(w5).astype(NBF)
    biases = np.stack([np.concatenate([b, b]) for b in
                       (b1, b2, b3, bt, b5)], 1).astype(f32)

    in_maps = []
    for core in range(8):
        bb, half = core // 2, core % 2
        A0 = 128 * half

        # xs: rows A0-4 .. A0+131 (136 = 68 pairs), partition (s, p, cin)
        xp = np.zeros((136, 262, 32), f32)
        lo, hi = max(0, A0 - 4), min(H, A0 + 132)
        xp[lo - (A0 - 4):hi - (A0 - 4), 1:W + 1] = dense[bb, lo:hi]
        xs = xp[:, 0:260].reshape(68, 2, 130, 2, 32)
        xs = xs.transpose(1, 3, 4, 0, 2).reshape(128, 68, 130)

        # m1P: rows A0-3 .. A0+130 (134), partition (p, ch)
        mp_ = np.zeros((134, 262), f32)
        lo, hi = max(0, A0 - 3), min(H, A0 + 131)
        mp_[lo - (A0 - 3):hi - (A0 - 3), 1:W + 1] = m1[bb, lo:hi]
        m1P = mp_[:, 0:260].reshape(134, 130, 2).transpose(2, 0, 1)
        m1P = np.broadcast_to(m1P[:, None], (2, 64, 134, 130)).reshape(
            128, 134, 130)

        # m1 plain: rows A0-1 .. A0+128 (130), idx j = col j-1
        mq = np.zeros((130, 258), f32)
        lo, hi = max(0, A0 - 1), min(H, A0 + 129)
        mq[lo - (A0 - 1):hi - (A0 - 1), 1:W + 1] = m1[bb, lo:hi]
        m1pl = np.broadcast_to(mq[None], (64, 130, 258))

        # m4P: h4 rows U0 .. U0+257, partition (c, ch), block x: v=2x+c
        U0 = 2 * A0
        mr = np.zeros((258, 512), f32)
        lo, hi = U0, min(513, U0 + 258)
        mr[0:hi - lo] = m4[bb, lo:hi, 0:512]
        m4P = mr.reshape(258, 256, 2).transpose(2, 0, 1)
        m4P = np.broadcast_to(m4P[:, None], (2, 64, 258, 256)).reshape(
            128, 258, 256)

        in_maps.append(dict(
            xs=np.ascontiguousarray(xs.astype(NBF)),
            m1P=np.ascontiguousarray(m1P.astype(NBF)),
            m1pl=np.ascontiguousarray(m1pl.astype(NBF)),
            m4P=np.ascontiguousarray(m4P.astype(NBF)),
            wc1=wc1, wc2=wc2, wc3=wc3, wcT=wcT, wc5=wc5, biases=biases,
        ))

    # ---- host column-510 patch data: strip conv for final col 510
    relu = lambda a: np.maximum(a, 0)
    xstrip = dense[:, :, 242:256]                      # cols 242..255
    m1s = m1[:, :, 242:256].astype(f32)[..., None]
    h = relu(_np_conv3(xstrip, w1) + b1) * m1s
    h = relu(_np_conv3(h, w2) + b2) * m1s
    h = relu(_np_conv3(h, w3) + b3) * m1s
    h4s = relu(_np_convT(h, wt_eff) + bt)              # [B, 513, 29, 64]
    m4s = m4[:, :, 484:513].astype(f32)[..., None]
    h4s = h4s * m4s
    col510 = np.zeros((B, 511, 64), f32)
    for dy in range(3):
        for dx in range(3):
            col510 += h4s[:, dy:dy + 511, 26 + dx] @ w5[dy, dx]
    col510 = relu(col510 + b5)
    _CACHE["col510"] = col510
    return in_maps


def _build_program():
    nc = bacc.Bacc("TRN2", target_bir_lowering=False, debug=False,
                   enable_asserts=True, num_devices=8)

    xs_d = nc.dram_tensor("xs", [128, 68, 130], BF16, kind="ExternalInput").ap()
    m1P_d = nc.dram_tensor("m1P", [128, 134, 130], BF16,
                           kind="ExternalInput").ap()
    m1pl_d = nc.dram_tensor("m1pl", [64, 130, 258], BF16,
                            kind="ExternalInput").ap()
    m4P_d = nc.dram_tensor("m4P", [128, 258, 256], BF16,
                           kind="ExternalInput").ap()
    wc1_d = nc.dram_tensor("wc1", [128, 8, 128], BF16, kind="ExternalInput").ap()
    wc2_d = nc.dram_tensor("wc2", [128, 6, 128], BF16, kind="ExternalInput").ap()
    wc3_d = nc.dram_tensor("wc3", [128, 6, 128], BF16, kind="ExternalInput").ap()
    wcT_d = nc.dram_tensor("wcT", [128, 3, 128], BF16, kind="ExternalInput").ap()
    wc5_d = nc.dram_tensor("wc5", [128, 6, 128], BF16, kind="ExternalInput").ap()
    bias_d = nc.dram_tensor("biases", [128, 5], F32, kind="ExternalInput").ap()
    out_d = nc.dram_tensor("out", [128, 256, 256], BF16,
                           kind="ExternalOutput").ap()

    with tile.TileContext(nc) as tc, ExitStack() as ctx:
        wp = ctx.enter_context(tc.tile_pool(name="wp", bufs=1))
        xp = ctx.enter_context(tc.tile_pool(name="xp", bufs=2))
        mp = ctx.enter_context(tc.tile_pool(name="mp", bufs=2))
        hp = ctx.enter_context(tc.tile_pool(name="hp", bufs=1))
        pp = ctx.enter_context(tc.tile_pool(name="pp", bufs=2, space="PSUM"))
        op = ctx.enter_context(tc.tile_pool(name="op", bufs=4))

        w1t = wp.tile([128, 8, 128], BF16, name="w1t")
        w2t = wp.tile([128, 6, 128], BF16, name="w2t")
        w3t = wp.tile([128, 6, 128], BF16, name="w3t")
        wTt = wp.tile([128, 3, 128], BF16, name="wTt")
        w5t = wp.tile([128, 6, 128], BF16, name="w5t")
        bt = wp.tile([128, 5], F32, name="bt")
        nc.sync.dma_start(w1t[:], wc1_d[:])
        nc.sync.dma_start(bt[:], bias_d[:])
        nc.scalar.dma_start(w2t[:], wc2_d[:])
        nc.scalar.dma_start(w3t[:], wc3_d[:])
        nc.gpsimd.dma_start(wTt[:], wcT_d[:])
        nc.gpsimd.dma_start(w5t[:], wc5_d[:])

        def load_chunk(c):
            x_ch = xp.tile([128, 12, 130], BF16, name="x_ch", tag="x")
            nc.sync.dma_start(x_ch[:], xs_d[:, 8 * c:8 * c + 12, :])
            m1P_ch = mp.tile([128, 22, 130], BF16, name="m1P_ch", tag="m1P")
            nc.sync.dma_start(m1P_ch[:], m1P_d[:, 16 * c:16 * c + 22, :])
            m1pl_ch = mp.tile([64, 18, 258], BF16, name="m1pl_ch", tag="m1pl")
            m4_ch = mp.tile([128, 34, 256], BF16, name="m4_ch", tag="m4")
            return x_ch, m1P_ch, m1pl_ch, m4_ch

        def conv1(x_ch, m1P_ch, h1):
            nc.gpsimd.memset(h1[0:64, :, 0:1], 0)
            nc.gpsimd.memset(h1[64:128, :, 128:130], 0)
            for j0 in range(0, 22, 4):
                g = min(4, 22 - j0)
                pc = pp.tile([128, 4, 128], F32, name="p1", tag="p23")
                for r in range(g):
                    o = j0 + r
                    k, opar = o // 2, o % 2
                    i = 0
                    for pd in range(2):
                        for delta in range(2):
                            nc.tensor.matmul(
                                pc[:, r:r + 1, :],
                                w1t[:, opar * 4 + pd * 2 + delta, :],
                                x_ch[:, k + pd, delta:delta + 128],
                                start=(i == 0), stop=(i == 3))
                            i += 1
                nc.scalar.activation(h1[64:128, j0:j0 + g, 0:128],
                                     pc[0:64, 0:g, :], RELU, bias=bt[0:64, 0:1])
                nc.scalar.activation(h1[0:64, j0:j0 + g, 1:129],
                                     pc[64:128, 0:g, :], RELU,
                                     bias=bt[64:128, 0:1])
                nc.vector.tensor_mul(h1[:, j0:j0 + g, :], h1[:, j0:j0 + g, :],
                                     m1P_ch[:, j0:j0 + g, :])

        def conv_mid(inp, wt_, m1P_ch, nrows, bias_ap, moff, h_out):
            nc.gpsimd.memset(h_out[0:64, :, 0:1], 0)
            nc.gpsimd.memset(h_out[64:128, :, 128:130], 0)
            for j0 in range(0, nrows, 4):
                g = min(4, nrows - j0)
                pc = pp.tile([128, 4, 128], F32, name="p23", tag="p23")
                i = 0
                for dy in range(3):
                    for delta in range(2):
                        nc.tensor.matmul(
                            pc[:, 0:g, :], wt_[:, dy * 2 + delta, :],
                            inp[:, j0 + dy:j0 + dy + g, delta:delta + 128],
                            start=(i == 0), stop=(i == 5))
                        i += 1
                nc.scalar.activation(h_out[64:128, j0:j0 + g, 0:128],
                                     pc[0:64, 0:g, :], RELU, bias=bias_ap[0:64])
                nc.scalar.activation(h_out[0:64, j0:j0 + g, 1:129],
                                     pc[64:128, 0:g, :], RELU,
                                     bias=bias_ap[64:128])
                nc.vector.tensor_mul(
                    h_out[:, j0:j0 + g, :], h_out[:, j0:j0 + g, :],
                    m1P_ch[:, j0 + moff:j0 + moff + g, :])

        def conv3_plain(h2, m1pl_ch, h3):
            nc.gpsimd.memset(h3[0:64, :, 0:1], 0)
            for j0 in range(0, 18, 4):
                g = min(4, 18 - j0)
                pc = pp.tile([128, 4, 128], F32, name="p23", tag="p23")
                i = 0
                for dy in range(3):
                    for delta in range(2):
                        nc.tensor.matmul(
                            pc[:, 0:g, :], w3t[:, dy * 2 + delta, :],
                            h2[:, j0 + dy:j0 + dy + g, delta:delta + 128],
                            start=(i == 0), stop=(i == 5))
                        i += 1
                nc.scalar.activation(h3[0:64, j0:j0 + g, 1:257:2],
                                     pc[0:64, 0:g, :], RELU, bias=bt[0:64, 2:3])
                nc.scalar.activation(h3[0:64, j0:j0 + g, 2:258:2],
                                     pc[64:128, 0:g, :], RELU,
                                     bias=bt[64:128, 2:3])
                nc.vector.tensor_mul(h3[0:64, j0:j0 + g, 1:257],
                                     h3[0:64, j0:j0 + g, 1:257],
                                     m1pl_ch[:, j0:j0 + g, 1:257])
                nc.vector.tensor_copy(h3[64:128, j0:j0 + g, 0:256],
                                      h3[0:64, j0:j0 + g, 1:257])

        def convT_part(h3, h4, m4_ch, u0s, last=False):
            for u0 in u0s:
                yl = u0 // 2 + 1
                pc = pp.tile([128, 2, 256], F32, name="pT", tag="pT", bufs=3)
                nc.tensor.matmul(pc[:, 0:1, :], wTt[:, 0, :],
                                 h3[:, yl, 0:256], start=True, stop=False)
                nc.tensor.matmul(pc[:, 0:1, :], wTt[:, 2, :],
                                 h3[:, yl - 1, 0:256], start=False, stop=True)
                nc.tensor.matmul(pc[:, 1:2, :], wTt[:, 1, :],
                                 h3[:, yl, 0:256], start=True, stop=True)
                if last:
                    nc.scalar.activation(h4[:, u0:u0 + 2, 0:256], pc[:],
                                         RELU, bias=bt[:, 3:4])
                else:
                    nc.vector.tensor_scalar(h4[:, u0:u0 + 2, 0:256], pc[:],
                                            bt[:, 3:4], 0.0, ADD, MAX)
                nc.vector.tensor_mul(h4[:, u0:u0 + 2, 0:256],
                                     h4[:, u0:u0 + 2, 0:256],
                                     m4_ch[:, u0:u0 + 2, :])

        def conv5_part(h4, c, jo0s):
            for jo0 in jo0s:
                pc = pp.tile([128, 2, 256], F32, name="p5", tag="p5", bufs=3)
                i = 0
                for dy in range(3):
                    for delta in range(2):
                        nc.tensor.matmul(
                            pc[:], w5t[:, dy * 2 + delta, :],
                            h4[:, jo0 + dy:jo0 + dy + 2, delta:delta + 256],
                            start=(i == 0), stop=(i == 5))
                        i += 1
                out_sb = op.tile([128, 2, 256], BF16, name="out_sb", tag="o")
                nc.scalar.activation(out_sb[:], pc[:], RELU, bias=bt[:, 4:5])
                nc.gpsimd.dma_start(out_d[:, 32 * c + jo0:32 * c + jo0 + 2, :],
                                    out_sb[:])

        prev = None
        for it in range(NCH + 1):
            if it < NCH:
                x_ch, m1P_ch, m1pl_ch, m4_ch = load_chunk(it)
                h1 = hp.tile([128, 22, 130], BF16, name="h1", tag="h1")
                h2 = hp.tile([128, 20, 130], BF16, name="h2", tag="h2")
                h3 = hp.tile([128, 18, 258], BF16, name="h3", tag="h3", bufs=2)
                h4 = hp.tile([128, 34, 258], BF16, name="h4", tag="h4", bufs=2)
                nc.gpsimd.memset(h4[:, :, 256:258], 0)
                conv1(x_ch, m1P_ch, h1)
            if prev is not None:
                h3p, h4p, m4p, cp = prev
                convT_part(h3p, h4p, m4p, range(0, 18, 2), last=(it == NCH))
            if it < NCH:
                nc.gpsimd.dma_start(m4_ch[:], m4P_d[:, 32 * it:32 * it + 34, :])
                nc.scalar.dma_start(m1pl_ch[:],
                                    m1pl_d[:, 16 * it:16 * it + 18, :])
                conv_mid(h1, w2t, m1P_ch, 20, bt[:, 1:2], 1, h2)
            if prev is not None:
                convT_part(h3p, h4p, m4p, range(18, 34, 2),
                           last=(it == NCH))
                conv5_part(h4p, cp, range(0, 16, 2))
            if it < NCH:
                conv3_plain(h2, m1pl_ch, h3)
            if prev is not None:
                conv5_part(h4p, cp, range(16, 32, 2))
            if it < NCH:
                prev = (h3, h4, m4_ch, it)
            else:
                prev = None

    nc.compile()
    return nc


def kernel(**inputs):
    features = np.asarray(inputs["features"], np.float32)
    coors = np.asarray(inputs["coors"], np.int32)
    args = [np.asarray(inputs[k], np.float32) for k in
            ("w1", "b1", "w2", "b2", "w3", "b3", "wt", "bt", "w5", "b5")]
    in_maps = _host_prep(features, coors, *args)
    if "nc" not in _CACHE:
        _CACHE["nc"] = _build_program()
    res = run_bass_kernel_spmd(_CACHE["nc"], in_maps,
                               core_ids=list(range(8)), trace=False)
    full = np.zeros((B, 511, 511, 64), np.float32)
    for core in range(8):
        o = np.asarray(res.results[core]["out"], dtype=np.float32)
        o = o.reshape(2, 64, 256, 256)            # [q, ch, row, blk]
        rows = np.zeros((256, 512, 64), np.float32)
        rows[:, 0::2] = o[0].transpose(1, 2, 0)
        rows[:, 1::2] = o[1].transpose(1, 2, 0)
        bb, half = core // 2, core % 2
        if half == 0:
            full[bb, 0:256] = rows[:, 0:511]
        else:
            full[bb, 256:511] = rows[0:255, 0:511]
    full[:, :, 510, :] = _CACHE["col510"]
    return full


# revision 16
# speedup vs baseline: 1.0388x; 1.0164x over previous
"""Trainium2 Bass kernel for the sparse-conv network (nn_ExampleNet).

Parity-packed (space-to-depth) formulation: activations live in SBUF as
[128 partitions = 64ch x column-parity], so every 3x3 conv is 6 matmuls of
[K=128, M=128] per 4 output rows (75% PE util) using VALID-style quadrant
weights with alternating block alignment between layers.  All activations/
weights bf16 (fp32 PSUM accumulation), masks bf16.  8-way SPMD: 4 batches x
2 row-halves; 8 chunks of 16 h3-rows per core.  convT emits h4 directly in
parity layout (3 matmuls / 2 rows); conv5 consumes it with delta in {0,1}.
The single h4 column v=512 is dropped on device (psum width) and final
output column 510 is recomputed on the host.
"""
from contextlib import ExitStack

import numpy as np
import ml_dtypes

import concourse.bacc as bacc
import concourse.mybir as mybir
import concourse.tile as tile
from concourse.bass_utils import run_bass_kernel_spmd

F32 = mybir.dt.float32
BF16 = mybir.dt.bfloat16
RELU = mybir.ActivationFunctionType.Relu
ADD = mybir.AluOpType.add
MAX = mybir.AluOpType.max

B, H, W = 4, 256, 256
NCH = 8            # chunks per core
R3 = 16            # h3 rows produced per chunk
NBF = ml_dtypes.bfloat16

_CACHE = {}


def _np_conv3(xp, w):
    Bc, Hc, Wc, Ci = xp.shape
    xpad = np.zeros((Bc, Hc + 2, Wc + 2, Ci), np.float32)
    xpad[:, 1:-1, 1:-1] = xp
    out = np.zeros((Bc, Hc, Wc, w.shape[3]), np.float32)
    for dy in range(3):
        for dx in range(3):
            out += xpad[:, dy:dy + Hc, dx:dx + Wc] @ w[dy, dx]
    return out


def _np_convT(xp, w):
    Bc, Hc, Wc, Ci = xp.shape
    out = np.zeros((Bc, 2 * Hc + 1, 2 * Wc + 1, w.shape[3]), np.float32)
    for dy in range(3):
        for dx in range(3):
            out[:, dy:dy + 2 * Hc:2, dx:dx + 2 * Wc:2] += xp @ w[dy, dx]
    return out


def _quad_weights(w):
    """conv with SAME/VALID quadrant packing: mats[(dy, delta)] [128,128]:
    [cin+64p, cout+64q] = w[dy, 2*delta+p-q] when 0<=dx<=2."""
    cin, cout = w.shape[2], w.shape[3]
    out = np.zeros((128, 6, 128), np.float32)
    for dy in range(3):
        for delta in range(2):
            for p in range(2):
                for q in range(2):
                    dx = 2 * delta + p - q
                    if 0 <= dx <= 2:
                        out[64 * p:64 * p + cin, dy * 2 + delta,
                            64 * q:64 * q + cout] = w[dy, dx]
    return out


def _conv1_weights(w1):
    out = np.zeros((128, 8, 128), np.float32)
    for op in range(2):
        for pd in range(2):
            for delta in range(2):
                idx = op * 4 + pd * 2 + delta
                for s in range(2):
                    dy = 2 * pd + s - op
                    if not (0 <= dy <= 2):
                        continue
                    for p in range(2):
                        for q in range(2):
                            dx = 2 * delta + p - q
                            if 0 <= dx <= 2:
                                out[64 * s + 32 * p:64 * s + 32 * p + 32, idx,
                                    64 * q:64 * q + 64] = w1[dy, dx]
    return out


def _convT_weights(wt_eff):
    out = np.zeros((128, 3, 128), np.float32)
    for dy in range(3):
        out[64:128, dy, 0:64] = wt_eff[dy, 0]
        out[64:128, dy, 64:128] = wt_eff[dy, 1]
        out[0:64, dy, 0:64] = wt_eff[dy, 2]
    return out


def _host_prep(features, coors, w1, b1, w2, b2, w3, b3, wt, bt, w5, b5):
    f32 = np.float32
    bi, yi, xi = coors[:, 0], coors[:, 1], coors[:, 2]
    flat = (bi.astype(np.int64) * H + yi) * W + xi
    dense = np.zeros((B * H * W, 32), f32)
    for c in range(32):
        dense[:, c] = np.bincount(flat, weights=features[:, c],
                                  minlength=B * H * W)
    dense = dense.reshape(B, H, W, 32)
    occ = np.bincount(flat, minlength=B * H * W).reshape(B, H, W) > 0
    m0p = np.zeros((B, H + 2, W + 2), bool)
    m0p[:, 1:-1, 1:-1] = occ
    m1 = np.zeros((B, H, W), bool)
    for dy in range(3):
        for dx in range(3):
            m1 |= m0p[:, dy:dy + H, dx:dx + W]
    m4 = np.zeros((B, 2 * H + 1, 2 * W + 1), bool)
    for dy in range(3):
        for dx in range(3):
            m4[:, dy:dy + 2 * H - 1:2, dx:dx + 2 * W - 1:2] |= m1

    wt_eff = wt[::-1, ::-1]
    wc1 = _conv1_weights(w1).astype(NBF)
    wc2 = _quad_weights(w2).astype(NBF)
    wc3 = _quad_weights(w3).astype(NBF)
    wcT = _convT_weights(wt_eff).astype(NBF)
    wc5 = _quad_weights(w5).astype(NBF)
    biases = np.stack([np.concatenate([b, b]) for b in
                       (b1, b2, b3, bt, b5)], 1).astype(f32)

    in_maps = []
    for core in range(8):
        bb, half = core // 2, core % 2
        A0 = 128 * half

        # xs: rows A0-4 .. A0+131 (136 = 68 pairs), partition (s, p, cin)
        xp = np.zeros((136, 262, 32), f32)
        lo, hi = max(0, A0 - 4), min(H, A0 + 132)
        xp[lo - (A0 - 4):hi - (A0 - 4), 1:W + 1] = dense[bb, lo:hi]
        xs = xp[:, 0:260].reshape(68, 2, 130, 2, 32)
        xs = xs.transpose(1, 3, 4, 0, 2).reshape(128, 68, 130)

        # m1P: rows A0-3 .. A0+130 (134), partition (p, ch)
        mp_ = np.zeros((134, 262), f32)
        lo, hi = max(0, A0 - 3), min(H, A0 + 131)
        mp_[lo - (A0 - 3):hi - (A0 - 3), 1:W + 1] = m1[bb, lo:hi]
        m1P = mp_[:, 0:260].reshape(134, 130, 2).transpose(2, 0, 1)
        m1P = np.broadcast_to(m1P[:, None], (2, 64, 134, 130)).reshape(
            128, 134, 130)

        # m1 plain: rows A0-1 .. A0+128 (130), idx j = col j-1
        mq = np.zeros((130, 258), f32)
        lo, hi = max(0, A0 - 1), min(H, A0 + 129)
        mq[lo - (A0 - 1):hi - (A0 - 1), 1:W + 1] = m1[bb, lo:hi]
        m1pl = np.broadcast_to(mq[None], (64, 130, 258))

        # m4P: h4 rows U0 .. U0+257, partition (c, ch), block x: v=2x+c
        U0 = 2 * A0
        mr = np.zeros((258, 512), f32)
        lo, hi = U0, min(513, U0 + 258)
        mr[0:hi - lo] = m4[bb, lo:hi, 0:512]
        m4P = mr.reshape(258, 256, 2).transpose(2, 0, 1)
        m4P = np.broadcast_to(m4P[:, None], (2, 64, 258, 256)).reshape(
            128, 258, 256)

        in_maps.append(dict(
            xs=np.ascontiguousarray(xs.astype(NBF)),
            m1P=np.ascontiguousarray(m1P.astype(NBF)),
            m1pl=np.ascontiguousarray(m1pl.astype(NBF)),
            m4P=np.ascontiguousarray(m4P.astype(NBF)),
            wc1=wc1, wc2=wc2, wc3=wc3, wcT=wcT, wc5=wc5, biases=biases,
        ))

    # ---- host column-510 patch data: strip conv for final col 510
    relu = lambda a: np.maximum(a, 0)
    xstrip = dense[:, :, 242:256]                      # cols 242..255
    m1s = m1[:, :, 242:256].astype(f32)[..., None]
    h = relu(_np_conv3(xstrip, w1) + b1) * m1s
    h = relu(_np_conv3(h, w2) + b2) * m1s
    h = relu(_np_conv3(h, w3) + b3) * m1s
    h4s = relu(_np_convT(h, wt_eff) + bt)              # [B, 513, 29, 64]
    m4s = m4[:, :, 484:513].astype(f32)[..., None]
    h4s = h4s * m4s
    col510 = np.zeros((B, 511, 64), f32)
    for dy in range(3):
        for dx in range(3):
            col510 += h4s[:, dy:dy + 511, 26 + dx] @ w5[dy, dx]
    col510 = relu(col510 + b5)
    _CACHE["col510"] = col510
    return in_maps


def _build_program():
    nc = bacc.Bacc("TRN2", target_bir_lowering=False, debug=False,
                   enable_asserts=True, num_devices=8)

    xs_d = nc.dram_tensor("xs", [128, 68, 130], BF16, kind="ExternalInput").ap()
    m1P_d = nc.dram_tensor("m1P", [128, 134, 130], BF16,
                           kind="ExternalInput").ap()
    m1pl_d = nc.dram_tensor("m1pl", [64, 130, 258], BF16,
                            kind="ExternalInput").ap()
    m4P_d = nc.dram_tensor("m4P", [128, 258, 256], BF16,
                           kind="ExternalInput").ap()
    wc1_d = nc.dram_tensor("wc1", [128, 8, 128], BF16, kind="ExternalInput").ap()
    wc2_d = nc.dram_tensor("wc2", [128, 6, 128], BF16, kind="ExternalInput").ap()
    wc3_d = nc.dram_tensor("wc3", [128, 6, 128], BF16, kind="ExternalInput").ap()
    wcT_d = nc.dram_tensor("wcT", [128, 3, 128], BF16, kind="ExternalInput").ap()
    wc5_d = nc.dram_tensor("wc5", [128, 6, 128], BF16, kind="ExternalInput").ap()
    bias_d = nc.dram_tensor("biases", [128, 5], F32, kind="ExternalInput").ap()
    out_d = nc.dram_tensor("out", [128, 256, 256], BF16,
                           kind="ExternalOutput").ap()

    with tile.TileContext(nc) as tc, ExitStack() as ctx:
        wp = ctx.enter_context(tc.tile_pool(name="wp", bufs=1))
        xp = ctx.enter_context(tc.tile_pool(name="xp", bufs=2))
        mp = ctx.enter_context(tc.tile_pool(name="mp", bufs=2))
        hp = ctx.enter_context(tc.tile_pool(name="hp", bufs=1))
        pp = ctx.enter_context(tc.tile_pool(name="pp", bufs=2, space="PSUM"))
        op = ctx.enter_context(tc.tile_pool(name="op", bufs=4))

        w1t = wp.tile([128, 8, 128], BF16, name="w1t")
        w2t = wp.tile([128, 6, 128], BF16, name="w2t")
        w3t = wp.tile([128, 6, 128], BF16, name="w3t")
        wTt = wp.tile([128, 3, 128], BF16, name="wTt")
        w5t = wp.tile([128, 6, 128], BF16, name="w5t")
        bt = wp.tile([128, 5], F32, name="bt")
        nc.sync.dma_start(w1t[:], wc1_d[:])
        nc.sync.dma_start(bt[:], bias_d[:])
        nc.scalar.dma_start(w2t[:], wc2_d[:])
        nc.scalar.dma_start(w3t[:], wc3_d[:])
        nc.gpsimd.dma_start(wTt[:], wcT_d[:])
        nc.gpsimd.dma_start(w5t[:], wc5_d[:])

        def load_chunk(c):
            x_ch = xp.tile([128, 12, 130], BF16, name="x_ch", tag="x")
            nc.sync.dma_start(x_ch[:], xs_d[:, 8 * c:8 * c + 12, :])
            m1P_ch = mp.tile([128, 22, 130], BF16, name="m1P_ch", tag="m1P")
            nc.sync.dma_start(m1P_ch[:, 0:8, :], m1P_d[:, 16 * c:16 * c + 8, :])
            nc.sync.dma_start(m1P_ch[:, 8:22, :],
                              m1P_d[:, 16 * c + 8:16 * c + 22, :])
            m1pl_ch = mp.tile([64, 18, 258], BF16, name="m1pl_ch", tag="m1pl")
            m4_ch = mp.tile([128, 34, 256], BF16, name="m4_ch", tag="m4")
            return x_ch, m1P_ch, m1pl_ch, m4_ch

        def conv1(x_ch, m1P_ch, h1):
            nc.gpsimd.memset(h1[0:64, :, 0:1], 0)
            nc.gpsimd.memset(h1[64:128, :, 128:130], 0)
            for j0 in range(0, 22, 4):
                g = min(4, 22 - j0)
                pc = pp.tile([128, 4, 128], F32, name="p1", tag="p23")
                for r in range(g):
                    o = j0 + r
                    k, opar = o // 2, o % 2
                    i = 0
                    for pd in range(2):
                        for delta in range(2):
                            nc.tensor.matmul(
                                pc[:, r:r + 1, :],
                                w1t[:, opar * 4 + pd * 2 + delta, :],
                                x_ch[:, k + pd, delta:delta + 128],
                                start=(i == 0), stop=(i == 3))
                            i += 1
                nc.scalar.activation(h1[64:128, j0:j0 + g, 0:128],
                                     pc[0:64, 0:g, :], RELU, bias=bt[0:64, 0:1])
                nc.scalar.activation(h1[0:64, j0:j0 + g, 1:129],
                                     pc[64:128, 0:g, :], RELU,
                                     bias=bt[64:128, 0:1])
                nc.vector.tensor_mul(h1[:, j0:j0 + g, :], h1[:, j0:j0 + g, :],
                                     m1P_ch[:, j0:j0 + g, :])

        def conv_mid(inp, wt_, m1P_ch, nrows, bias_ap, moff, h_out):
            nc.gpsimd.memset(h_out[0:64, :, 0:1], 0)
            nc.gpsimd.memset(h_out[64:128, :, 128:130], 0)
            for j0 in range(0, nrows, 4):
                g = min(4, nrows - j0)
                pc = pp.tile([128, 4, 128], F32, name="p23", tag="p23")
                i = 0
                for dy in range(3):
                    for delta in range(2):
                        nc.tensor.matmul(
                            pc[:, 0:g, :], wt_[:, dy * 2 + delta, :],
                            inp[:, j0 + dy:j0 + dy + g, delta:delta + 128],
                            start=(i == 0), stop=(i == 5))
                        i += 1
                nc.scalar.activation(h_out[64:128, j0:j0 + g, 0:128],
                                     pc[0:64, 0:g, :], RELU, bias=bias_ap[0:64])
                nc.scalar.activation(h_out[0:64, j0:j0 + g, 1:129],
                                     pc[64:128, 0:g, :], RELU,
                                     bias=bias_ap[64:128])
                nc.vector.tensor_mul(
                    h_out[:, j0:j0 + g, :], h_out[:, j0:j0 + g, :],
                    m1P_ch[:, j0 + moff:j0 + moff + g, :])

        def conv3_plain(h2, m1pl_ch, h3):
            nc.gpsimd.memset(h3[0:64, :, 0:1], 0)
            for j0 in range(0, 18, 4):
                g = min(4, 18 - j0)
                pc = pp.tile([128, 4, 128], F32, name="p23", tag="p23")
                i = 0
                for dy in range(3):
                    for delta in range(2):
                        nc.tensor.matmul(
                            pc[:, 0:g, :], w3t[:, dy * 2 + delta, :],
                            h2[:, j0 + dy:j0 + dy + g, delta:delta + 128],
                            start=(i == 0), stop=(i == 5))
                        i += 1
                nc.scalar.activation(h3[0:64, j0:j0 + g, 1:257:2],
                                     pc[0:64, 0:g, :], RELU, bias=bt[0:64, 2:3])
                nc.scalar.activation(h3[0:64, j0:j0 + g, 2:258:2],
                                     pc[64:128, 0:g, :], RELU,
                                     bias=bt[64:128, 2:3])
                nc.vector.tensor_mul(h3[0:64, j0:j0 + g, 1:257],
                                     h3[0:64, j0:j0 + g, 1:257],
                                     m1pl_ch[:, j0:j0 + g, 1:257])
                nc.vector.tensor_copy(h3[64:128, j0:j0 + g, 0:256],
                                      h3[0:64, j0:j0 + g, 1:257])

        def convT_part(h3, h4, m4_ch, u0s, last=False):
            for u0 in u0s:
                yl = u0 // 2 + 1
                pc = pp.tile([128, 2, 256], F32, name="pT", tag="pT", bufs=3)
                nc.tensor.matmul(pc[:, 0:1, :], wTt[:, 0, :],
                                 h3[:, yl, 0:256], start=True, stop=False)
                nc.tensor.matmul(pc[:, 0:1, :], wTt[:, 2, :],
                                 h3[:, yl - 1, 0:256], start=False, stop=True)
                nc.tensor.matmul(pc[:, 1:2, :], wTt[:, 1, :],
                                 h3[:, yl, 0:256], start=True, stop=True)
                if last:
                    nc.scalar.activation(h4[:, u0:u0 + 2, 0:256], pc[:],
                                         RELU, bias=bt[:, 3:4])
                else:
                    nc.vector.tensor_scalar(h4[:, u0:u0 + 2, 0:256], pc[:],
                                            bt[:, 3:4], 0.0, ADD, MAX)
                nc.vector.tensor_mul(h4[:, u0:u0 + 2, 0:256],
                                     h4[:, u0:u0 + 2, 0:256],
                                     m4_ch[:, u0:u0 + 2, :])

        def conv5_part(h4, c, jo0s):
            for jo0 in jo0s:
                pc = pp.tile([128, 2, 256], F32, name="p5", tag="p5", bufs=3)
                i = 0
                for dy in range(3):
                    for delta in range(2):
                        nc.tensor.matmul(
                            pc[:], w5t[:, dy * 2 + delta, :],
                            h4[:, jo0 + dy:jo0 + dy + 2, delta:delta + 256],
                            start=(i == 0), stop=(i == 5))
                        i += 1
                out_sb = op.tile([128, 2, 256], BF16, name="out_sb", tag="o")
                nc.scalar.activation(out_sb[:], pc[:], RELU, bias=bt[:, 4:5])
                nc.gpsimd.dma_start(out_d[:, 32 * c + jo0:32 * c + jo0 + 2, :],
                                    out_sb[:])

        prev = None
        for it in range(NCH + 1):
            if it < NCH:
                x_ch, m1P_ch, m1pl_ch, m4_ch = load_chunk(it)
                h1 = hp.tile([128, 22, 130], BF16, name="h1", tag="h1")
                h2 = hp.tile([128, 20, 130], BF16, name="h2", tag="h2")
                h3 = hp.tile([128, 18, 258], BF16, name="h3", tag="h3", bufs=2)
                h4 = hp.tile([128, 34, 258], BF16, name="h4", tag="h4", bufs=2)
                nc.gpsimd.memset(h4[:, :, 256:258], 0)
                conv1(x_ch, m1P_ch, h1)
            if prev is not None:
                h3p, h4p, m4p, cp = prev
                if it == NCH:
                    convT_part(h3p, h4p, m4p, range(0, 6, 2))
                    for k, u0 in enumerate(range(6, 34, 2)):
                        convT_part(h3p, h4p, m4p, [u0])
                        if 2 * k <= 26:
                            conv5_part(h4p, cp, [2 * k])
                    conv5_part(h4p, cp, range(28, 32, 2))
                else:
                    convT_part(h3p, h4p, m4p, range(0, 18, 2))
            if it < NCH:
                nc.gpsimd.dma_start(m4_ch[:], m4P_d[:, 32 * it:32 * it + 34, :])
                nc.scalar.dma_start(m1pl_ch[:],
                                    m1pl_d[:, 16 * it:16 * it + 18, :])
                conv_mid(h1, w2t, m1P_ch, 20, bt[:, 1:2], 1, h2)
            if prev is not None and it < NCH:
                convT_part(h3p, h4p, m4p, range(18, 34, 2))
                conv5_part(h4p, cp, range(0, 16, 2))
            if it < NCH:
                conv3_plain(h2, m1pl_ch, h3)
            if prev is not None and it < NCH:
                conv5_part(h4p, cp, range(16, 32, 2))
            if it < NCH:
                prev = (h3, h4, m4_ch, it)
            else:
                prev = None

    nc.compile()
    return nc


def kernel(**inputs):
    features = np.asarray(inputs["features"], np.float32)
    coors = np.asarray(inputs["coors"], np.int32)
    args = [np.asarray(inputs[k], np.float32) for k in
            ("w1", "b1", "w2", "b2", "w3", "b3", "wt", "bt", "w5", "b5")]
    in_maps = _host_prep(features, coors, *args)
    if "nc" not in _CACHE:
        _CACHE["nc"] = _build_program()
    res = run_bass_kernel_spmd(_CACHE["nc"], in_maps,
                               core_ids=list(range(8)), trace=False)
    full = np.zeros((B, 511, 511, 64), np.float32)
    for core in range(8):
        o = np.asarray(res.results[core]["out"], dtype=np.float32)
        o = o.reshape(2, 64, 256, 256)            # [q, ch, row, blk]
        rows = np.zeros((256, 512, 64), np.float32)
        rows[:, 0::2] = o[0].transpose(1, 2, 0)
        rows[:, 1::2] = o[1].transpose(1, 2, 0)
        bb, half = core // 2, core % 2
        if half == 0:
            full[bb, 0:256] = rows[:, 0:511]
        else:
            full[bb, 256:511] = rows[0:255, 0:511]
    full[:, :, 510, :] = _CACHE["col510"]
    return full
